# revision 3
# baseline (speedup 1.0000x reference)
"""HGT (heterogeneous graph transformer) kernel for 8 trn2 NeuronCores.

Design:
  - Host folds a_rel/m_rel/p_rel into per-src-type effective Wk/Wv (each node
    type is src of exactly one edge type), and sigmoid(skip) into Wa/ba.
  - Nodes sharded 8 ways (user 6250, movie 2500, review 25000 per core).
  - Review-dst edges (mr+ur merged) sharded by dst review shard; movie/user
    k|v (interleaved [*,512]) and user q AllGathered each layer (halo).
  - Review-src edges (ru) sharded by src review shard; per-core partial
    (den|msg) scattered into [8*6400, 264] buffer, ReduceScattered to owner.
  - Edges packed into 128-slot tiles with no dst group straddling a tile:
    segment softmax = exp (logits are tiny; max-subtract safely skipped) +
    on-device selection-matrix matmul (key equality) for den and msg sums,
    then indirect-DMA scatter (duplicate rows write identical values).
"""

import math
import numpy as np

try:
    import concourse  # noqa
except ImportError:
    import sys
    sys.path.insert(0, "/opt/trn_rl_repo")

from concourse import bacc, bass, mybir, tile
from concourse.bass import IndirectOffsetOnAxis
from concourse.bass_utils import run_bass_kernel_spmd
from concourse.masks import make_identity

P = 128
H, DH, HID, IN_DIM, OUT_DIM = 8, 32, 256, 768, 128
L = 2
NU_F, NM_F, NR_F = 50000, 20000, 200000
C = 8
NU, NM, NR = NU_F // C, NM_F // C, NR_F // C  # 6250, 2500, 25000
AG_BLK = NM + NU  # 8750 rows per core in the kv allgather (movie first)
UBLK = 6400       # padded user block in the ReduceScatter partial
B = 1             # edge tiles per indirect-DMA batch ([P,1] offsets only)
F32 = mybir.dt.float32
I32 = mybir.dt.int32
AF = mybir.ActivationFunctionType
ALU = mybir.AluOpType

LAST_RESULTS = None  # test.py reads exec_time_ns/profile from here


# ---------------------------------------------------------------- host prep

def _fold_weights(inp):
    """Fold a_rel/m_rel/p_rel into Wk/Wv per src type; sigmoid(skip) into Wa."""
    Wk, bk = inp["Wk"], inp["bk"]
    Wq, bq = inp["Wq"], inp["bq"]
    Wv, bv = inp["Wv"], inp["bv"]
    Wa, ba = inp["Wa"], inp["ba"]
    a_rel, m_rel, p_rel, skip = inp["a_rel"], inp["m_rel"], inp["p_rel"], inp["skip"]
    s_of_e = {0: 1, 1: 0, 2: 2}  # edge type -> src node type
    out = {}
    for l in range(L):
        for e in range(3):
            s = s_of_e[e]
            wk_eff = np.empty((HID, HID), np.float32)
            bk_eff = np.empty((HID,), np.float32)
            wv_eff = np.empty((HID, HID), np.float32)
            bv_eff = np.empty((HID,), np.float32)
            for h in range(H):
                sl = slice(h * DH, (h + 1) * DH)
                sc = float(p_rel[l, e, h]) / math.sqrt(DH)
                wk_eff[:, sl] = (Wk[l, s][:, sl] @ a_rel[l, e, h]) * sc
                bk_eff[sl] = (bk[l, s][sl] @ a_rel[l, e, h]) * sc
                wv_eff[:, sl] = Wv[l, s][:, sl] @ m_rel[l, e, h]
                bv_eff[sl] = bv[l, s][sl] @ m_rel[l, e, h]
            # kv interleaved: one gather fetches both k and v
            out[f"wkv_t{s}_l{l}"] = np.concatenate([wk_eff, wv_eff], 1)
            out[f"bkv_t{s}_l{l}"] = np.concatenate([bk_eff, bv_eff]).reshape(1, 512)
        for t in (0, 2):  # q only needed for user and review (movie never dst)
            out[f"wq_t{t}_l{l}"] = np.ascontiguousarray(Wq[l, t])
            out[f"bq_t{t}_l{l}"] = np.ascontiguousarray(bq[l, t]).reshape(1, HID)
        for t in range(3):
            g = 1.0 / (1.0 + math.exp(-float(skip[l, t])))
            out[f"omg_l{l}_t{t}"] = 1.0 - g  # python float, baked into program
            if t != 1:
                out[f"wa_t{t}_l{l}"] = np.ascontiguousarray(Wa[l, t]) * g
            out[f"ba_t{t}_l{l}"] = (np.ascontiguousarray(ba[l, t]) * g).reshape(1, HID)
    out["w1"] = np.ascontiguousarray(inp["W1"])
    out["b1"] = inp["b1"].reshape(1, HID).astype(np.float32)
    out["w2"] = np.ascontiguousarray(inp["W2"])
    out["b2"] = inp["b2"].reshape(1, OUT_DIM).astype(np.float32)
    return out


def _pack(group_ids, payload_cols, pad_vals, dtypes):
    """Pack edges (sorted by group) into 128-slot tiles; groups never straddle
    a tile. Returns list of [T, P] arrays (caller transposes after padding T)."""
    n = len(group_ids)
    if n == 0:
        return 0, [np.full((0, P), pv, dt) for pv, dt in zip(pad_vals, dtypes)]
    order = np.argsort(group_ids, kind="stable")
    g = group_ids[order]
    uniq, counts = np.unique(g, return_counts=True)
    ng = len(uniq)
    tile_id = np.empty(ng, np.int64)
    slot0 = np.empty(ng, np.int64)
    cur_t, fill = 0, 0
    cl = counts.tolist()
    for i in range(ng):
        c = cl[i]
        assert c <= P, f"group degree {c} > {P}"
        if fill + c > P:
            cur_t += 1
            fill = 0
        tile_id[i] = cur_t
        slot0[i] = fill
        fill += c
    T = cur_t + 1
    gi = np.repeat(np.arange(ng), counts)
    starts = np.cumsum(counts) - counts
    within = np.arange(n) - starts[gi]
    tid = tile_id[gi]
    slot = slot0[gi] + within
    outs = []
    for col, pv, dt in zip(payload_cols, pad_vals, dtypes):
        arr = np.full((T, P), pv, dtype=dt)
        arr[tid, slot] = col[order].astype(dt)
        outs.append(arr)
    return T, outs


def _equalize(per_core, pad_vals, dtypes):
    """Pad every core's [T,P] arrays to common T (multiple of B); -> [P,T]."""
    T = max(max(t for t, _ in per_core), 1)
    T = ((T + B - 1) // B) * B
    res = []
    for _, arrs in per_core:
        padded = []
        for a, pv, dt in zip(arrs, pad_vals, dtypes):
            full = np.full((T, P), pv, dtype=dt)
            full[: a.shape[0]] = a
            padded.append(np.ascontiguousarray(full.T))  # [P, T]
        res.append(padded)
    return T, res


def _prep_edges(inp):
    """Build per-core packed edge-tile index arrays for both edge phases."""
    src_mr, dst_mr = inp["src_mr"], inp["dst_mr"]
    src_ur, dst_ur = inp["src_ur"], inp["dst_ur"]
    src_ru, dst_ru = inp["src_ru"], inp["dst_ru"]

    # phase C: review-dst edges (mr type0 + ur type1), sharded by dst shard
    sm = (src_mr // NM) * AG_BLK + (src_mr % NM)
    su = (src_ur // NU) * AG_BLK + NM + (src_ur % NU)
    src_all = np.concatenate([sm, su]).astype(np.int64)
    dst_all = np.concatenate([dst_mr, dst_ur]).astype(np.int64)
    typ_all = np.concatenate(
        [np.zeros(len(sm), np.int64), np.ones(len(su), np.int64)])
    csp = []
    pvC = [0, 0, NR, -1.0, -1.0]
    dtC = [np.int32, np.int32, np.int32, np.float32, np.float32]
    for c in range(C):
        m = (dst_all // NR) == c
        dl = dst_all[m] % NR
        key = dl * 2 + typ_all[m]
        cols = [src_all[m], dl, dl, key.astype(np.float64), dl.astype(np.float64)]
        csp.append(_pack(dl, cols, pvC, dtC))
    T_C, cs = _equalize(csp, pvC, dtC)

    # phase D: ru edges (review->user), sharded by src shard
    s64, d64 = src_ru.astype(np.int64), dst_ru.astype(np.int64)
    flat = (d64 // NU) * UBLK + (d64 % NU)
    rup = []
    pvD = [0, 0, NU, -1.0]
    dtD = [np.int32, np.int32, np.int32, np.float32]
    for c in range(C):
        m = (s64 // NR) == c
        cols = [s64[m] % NR, d64[m], flat[m], flat[m].astype(np.float64)]
        rup.append(_pack(flat[m], cols, pvD, dtD))
    T_D, ru = _equalize(rup, pvD, dtD)
    return T_C, cs, T_D, ru


# ---------------------------------------------------------------- device

def _rows_of(n):
    return [(r0, min(P, n - r0)) for r0 in range(0, n, P)]


def build_program(T_C, T_D, omg):
    """Build the 8-core SPMD Bass program. omg: {(l,t): 1-sigmoid(skip)}."""
    nc = bacc.Bacc("TRN2", target_bir_lowering=False, debug=False,
                   enable_asserts=False, num_devices=C)
    RG = [list(range(C))]

    def din(name, shape, dt=F32):
        return nc.dram_tensor(name, list(shape), dt, kind="ExternalInput")

    def dint(name, shape, dt=F32, shared=False):
        return nc.dram_tensor(name, list(shape), dt, kind="Internal",
                              addr_space="Shared" if shared else "Local")

    x_u = din("x_u", (NU, IN_DIM))
    x_m = din("x_m", (NM, IN_DIM))
    x_r = din("x_r", (NR, IN_DIM))
    cs_names = ["cs_src", "cs_qi", "cs_dst", "cs_key", "cs_dkey"]
    cs_dt = [I32, I32, I32, F32, F32]
    cs_d = [din(n, (P, T_C), d) for n, d in zip(cs_names, cs_dt)]
    ru_names = ["ru_src", "ru_qi", "ru_dst", "ru_key"]
    ru_dt = [I32, I32, I32, F32]
    ru_d = [din(n, (P, T_D), d) for n, d in zip(ru_names, ru_dt)]
    w1 = din("w1", (IN_DIM, HID))
    b1 = din("b1", (1, HID))
    w2 = din("w2", (HID, OUT_DIM))
    b2 = din("b2", (1, OUT_DIM))
    wd, bd = {}, {}
    for l in range(L):
        for s in range(3):
            wd[f"wkv_t{s}_l{l}"] = din(f"wkv_t{s}_l{l}", (HID, 512))
            bd[f"bkv_t{s}_l{l}"] = din(f"bkv_t{s}_l{l}", (1, 512))
        for t in (0, 2):
            wd[f"wq_t{t}_l{l}"] = din(f"wq_t{t}_l{l}", (HID, HID))
            bd[f"bq_t{t}_l{l}"] = din(f"bq_t{t}_l{l}", (1, HID))
            wd[f"wa_t{t}_l{l}"] = din(f"wa_t{t}_l{l}", (HID, HID))
            bd[f"ba_t{t}_l{l}"] = din(f"ba_t{t}_l{l}", (1, HID))
        bd[f"ba_t1_l{l}"] = din(f"ba_t1_l{l}", (1, HID))
    y_u = nc.dram_tensor("y_u", [NU, OUT_DIM], F32, kind="ExternalOutput")
    y_m = nc.dram_tensor("y_m", [NM, OUT_DIM], F32, kind="ExternalOutput")
    y_r = nc.dram_tensor("y_r", [NR, OUT_DIM], F32, kind="ExternalOutput")

    # internal DRAM; xs[t][stage] for stage 0..2
    xs = {t: [dint(f"xs_t{t}_s{s}", (n, HID)) for s in range(L + 1)]
          for t, n in ((0, NU), (1, NM), (2, NR))}
    kv_own = [dint(f"kv_own_l{l}", (AG_BLK, 512)) for l in range(L)]
    qu_own = [dint(f"qu_own_l{l}", (NU, HID)) for l in range(L)]
    kv_src = [dint(f"kv_src_l{l}", (C * AG_BLK, 512), shared=True) for l in range(L)]
    q_uf = [dint(f"q_uf_l{l}", (NU_F, HID), shared=True) for l in range(L)]
    kv_ru = [dint(f"kv_ru_l{l}", (NR, 512)) for l in range(L)]
    q_r = [dint(f"q_r_l{l}", (NR, HID)) for l in range(L)]
    outs_r = [dint(f"outs_r_l{l}", (NR + 1, HID)) for l in range(L)]
    part_u = [dint(f"part_u_l{l}", (C * UBLK, 264)) for l in range(L)]
    red_u = [dint(f"red_u_l{l}", (UBLK, 264)) for l in range(L)]

    with tile.TileContext(nc) as tc:
        from contextlib import ExitStack
        _stk = ExitStack()
        wp = _stk.enter_context(tc.tile_pool(name="wp", bufs=1))

        def mk(shape, dt, name):
            return wp.tile(shape, dt, tag=name, name=name)

        ident = mk([P, P], F32, "ident")
        make_identity(nc, ident[:, :])
        # persistent SBUF: edge indices
        cs_sb = []
        for n, d, dr in zip(cs_names, cs_dt, cs_d):
            t_ = mk([P, T_C], d, n + "_sb")
            nc.sync.dma_start(t_[:], dr.ap()[:, :])
            cs_sb.append(t_)
        ru_sb = []
        for n, d, dr in zip(ru_names, ru_dt, ru_d):
            t_ = mk([P, T_D], d, n + "_sb")
            nc.sync.dma_start(t_[:], dr.ap()[:, :])
            ru_sb.append(t_)

        def load_w(dr, in_dim, out_w, name):
            ts = []
            for cch in range(in_dim // P):
                t_ = mk([P, out_w], F32, f"{name}_c{cch}")
                nc.sync.dma_start(t_[:], dr.ap()[cch * P:(cch + 1) * P, :])
                ts.append(t_)
            return ts

        def load_b(dr, w, name):
            t_ = mk([P, w], F32, name)
            nc.sync.dma_start(t_[:], dr.ap()[0:1, :].to_broadcast([P, w]))
            return t_

        w1_s = load_w(w1, IN_DIM, HID, "w1s")
        b1_s = load_b(b1, HID, "b1s")
        w2_s = load_w(w2, HID, OUT_DIM, "w2s")
        b2_s = load_b(b2, OUT_DIM, "b2s")
        ws, bs = {}, {}
        for k, dr in wd.items():
            ws[k] = load_w(dr, HID, 512 if k.startswith("wkv") else HID, k + "s")
        for k, dr in bd.items():
            bs[k] = load_b(dr, 512 if k.startswith("bkv") else HID, k + "s")

        sb = _stk.enter_context(tc.tile_pool(name="sb", bufs=2))
        pp = _stk.enter_context(tc.tile_pool(name="pp", bufs=2, space="PSUM"))

        zt = mk([P, 8, 264], F32, "zt")
        nc.vector.memset(zt[:], 0.0)

        def memset_dram(dr, nrows, w, tag):
            nfull = (nrows // P) * P
            v = dr.ap()[0:nfull, :].rearrange("(p a) f -> p a f", p=P)
            a_tot = nfull // P
            a0 = 0
            while a0 < a_tot:
                aa = min(8, a_tot - a0)
                nc.sync.dma_start(v[:, a0:a0 + aa, :],
                                  zt[:, 0:aa, 0:w])
                a0 += aa
            if nrows > nfull:
                r = nrows - nfull
                nc.sync.dma_start(dr.ap()[nfull:nrows, :], zt[0:r, 0, 0:w])

        def transposed(xt_ap, sz, nch, tag):
            outs = []
            for cch in range(nch):
                tp = pp.tile([P, P], F32, tag="tp")
                nc.tensor.transpose(
                    out=tp[:, 0:sz],
                    in_=xt_ap[0:sz, cch * P:(cch + 1) * P],
                    identity=ident[0:sz, 0:sz])
                ts = sb.tile([P, P], F32, tag=f"dts{cch}")
                nc.vector.tensor_copy(ts[:, 0:sz], tp[:, 0:sz])
                outs.append(ts)
            return outs

        def dense(x_dr, nrows, in_dim, jobs, tag):
            """jobs: (w_tiles, finish(ps_ap, r0, sz))"""
            nch = in_dim // P
            for r0, sz in _rows_of(nrows):
                xt = sb.tile([P, in_dim], F32, tag="dx")
                nc.sync.dma_start(xt[0:sz], x_dr.ap()[r0:r0 + sz, :])
                xT = transposed(xt, sz, nch, tag)
                for wt, finish in jobs:
                    ow = wt[0].shape[-1]
                    ps = pp.tile([P, ow], F32, tag="ps")
                    for cch in range(nch):
                        nc.tensor.matmul(out=ps[0:sz], lhsT=xT[cch][:, 0:sz],
                                         rhs=wt[cch][:],
                                         start=(cch == 0), stop=(cch == nch - 1))
                    finish(ps, r0, sz)

        def fin_store(bias_t, act, out_dr, off, ow, tag, alpha=0.0):
            def f(ps, r0, sz):
                ot = sb.tile([P, ow], F32, tag="do")
                nc.vector.tensor_add(ot[0:sz], ps[0:sz], bias_t[0:sz, :])
                if act is not None:
                    nc.scalar.activation(out=ot[0:sz], in_=ot[0:sz], func=act,
                                         alpha=alpha)
                nc.sync.dma_start(out_dr.ap()[off + r0: off + r0 + sz, :],
                                  ot[0:sz])
            return f

        # ---- phase 0: input MLP
        for t, x_dr, n in ((0, x_u, NU), (1, x_m, NM), (2, x_r, NR)):
            dense(x_dr, n, IN_DIM,
                  [(w1_s, fin_store(b1_s, AF.Lrelu, xs[t][0], 0, HID,
                                    f"p0t{t}", alpha=0.01))], f"p0t{t}")

        for l in range(L):
            # ---- phase A: kqv from own shards
            dense(xs[0][l], NU, HID, [
                (ws[f"wkv_t0_l{l}"], fin_store(bs[f"bkv_t0_l{l}"], None,
                                               kv_own[l], NM, 512, f"au{l}")),
                (ws[f"wq_t0_l{l}"], fin_store(bs[f"bq_t0_l{l}"], None,
                                              qu_own[l], 0, HID, f"aq{l}")),
            ], f"au{l}")
            dense(xs[1][l], NM, HID, [
                (ws[f"wkv_t1_l{l}"], fin_store(bs[f"bkv_t1_l{l}"], None,
                                               kv_own[l], 0, 512, f"am{l}")),
            ], f"am{l}")
            dense(xs[2][l], NR, HID, [
                (ws[f"wkv_t2_l{l}"], fin_store(bs[f"bkv_t2_l{l}"], None,
                                               kv_ru[l], 0, 512, f"ar{l}")),
                (ws[f"wq_t2_l{l}"], fin_store(bs[f"bq_t2_l{l}"], None,
                                              q_r[l], 0, HID, f"arq{l}")),
            ], f"ar{l}")

            # ---- phase B: halo allgathers
            nc.gpsimd.collective_compute(
                "AllGather", ALU.bypass, replica_groups=RG,
                ins=[kv_own[l].ap()], outs=[kv_src[l].ap()])
            nc.gpsimd.collective_compute(
                "AllGather", ALU.bypass, replica_groups=RG,
                ins=[qu_own[l].ap()], outs=[q_uf[l].ap()])

            memset_dram(outs_r[l], NR + 1, HID, f"z1{l}")
            memset_dram(part_u[l], C * UBLK, 264, f"z2{l}")

            # ---- phase C: review-dst edge tiles
            src_sb, qi_sb, dst_sb, key_sb, dkey_sb = cs_sb
            for tj in range(T_C):
                kvg = sb.tile([P, 512], F32, tag="kv")
                nc.gpsimd.indirect_dma_start(
                    out=kvg[:], out_offset=None, in_=kv_src[l].ap(),
                    in_offset=IndirectOffsetOnAxis(
                        ap=src_sb[:, tj:tj + 1], axis=0))
                qg = sb.tile([P, HID], F32, tag="q")
                nc.gpsimd.indirect_dma_start(
                    out=qg[:], out_offset=None, in_=q_r[l].ap(),
                    in_offset=IndirectOffsetOnAxis(
                        ap=qi_sb[:, tj:tj + 1], axis=0))
                kq = sb.tile([P, HID], F32, tag="kq")
                nc.vector.tensor_mul(kq[:], kvg[:, 0:HID], qg[:])
                lg = sb.tile([P, H], F32, tag="lg")
                nc.vector.tensor_reduce(
                    out=lg[:], in_=kq[:].rearrange("p (h d) -> p h d", h=H),
                    axis=mybir.AxisListType.X, op=ALU.add)
                ex = sb.tile([P, H], F32, tag="ex")
                nc.scalar.activation(out=ex[:], in_=lg[:], func=AF.Exp)
                kt_ps = pp.tile([P, P], F32, tag="tp")
                nc.tensor.transpose(
                    out=kt_ps[:],
                    in_=key_sb[:, tj:tj + 1].to_broadcast([P, P]),
                    identity=ident[:, :])
                kt = sb.tile([P, P], F32, tag="kt")
                nc.vector.tensor_copy(kt[:], kt_ps[:])
                sel = sb.tile([P, P], F32, tag="sel")
                nc.vector.tensor_tensor(
                    out=sel[:], in0=key_sb[:, tj:tj + 1].to_broadcast([P, P]),
                    in1=kt[:], op=ALU.is_equal)
                den = pp.tile([P, H], F32, tag="den")
                nc.tensor.matmul(out=den[:], lhsT=sel[:], rhs=ex[:],
                                 start=True, stop=True)
                rden = sb.tile([P, H], F32, tag="rdn")
                nc.vector.reciprocal(out=rden[:], in_=den[:])
                attn = sb.tile([P, H], F32, tag="at2")
                nc.vector.tensor_mul(attn[:], ex[:], rden[:])
                msg = sb.tile([P, HID], F32, tag="ms264")
                nc.vector.tensor_tensor(
                    out=msg[:].rearrange("p (h d) -> p h d", h=H),
                    in0=kvg[:, HID:512].rearrange("p (h d) -> p h d", h=H),
                    in1=attn[:].rearrange("p (h o) -> p h o", h=H)
                        .to_broadcast([P, H, DH]),
                    op=ALU.mult)
                dt_ps = pp.tile([P, P], F32, tag="tp")
                nc.tensor.transpose(
                    out=dt_ps[:],
                    in_=dkey_sb[:, tj:tj + 1].to_broadcast([P, P]),
                    identity=ident[:, :])
                dt_ = sb.tile([P, P], F32, tag="kt")
                nc.vector.tensor_copy(dt_[:], dt_ps[:])
                seld = sb.tile([P, P], F32, tag="sel")
                nc.vector.tensor_tensor(
                    out=seld[:],
                    in0=dkey_sb[:, tj:tj + 1].to_broadcast([P, P]),
                    in1=dt_[:], op=ALU.is_equal)
                msum = pp.tile([P, HID], F32, tag="ps")
                nc.tensor.matmul(out=msum[:], lhsT=seld[:], rhs=msg[:],
                                 start=True, stop=True)
                mo = sb.tile([P, HID], F32, tag="mo264")
                nc.vector.tensor_copy(mo[:], msum[:])
                nc.gpsimd.indirect_dma_start(
                    out=outs_r[l].ap(), in_=mo[:],
                    out_offset=IndirectOffsetOnAxis(
                        ap=dst_sb[:, tj:tj + 1], axis=0),
                    in_offset=None)

            # ---- phase D: ru edge tiles
            rsrc_sb, rqi_sb, rdst_sb, rkey_sb = ru_sb
            for tj in range(T_D):
                kvg = sb.tile([P, 512], F32, tag="kv")
                nc.gpsimd.indirect_dma_start(
                    out=kvg[:], out_offset=None, in_=kv_ru[l].ap(),
                    in_offset=IndirectOffsetOnAxis(
                        ap=rsrc_sb[:, tj:tj + 1], axis=0))
                qg = sb.tile([P, HID], F32, tag="q")
                nc.gpsimd.indirect_dma_start(
                    out=qg[:], out_offset=None, in_=q_uf[l].ap(),
                    in_offset=IndirectOffsetOnAxis(
                        ap=rqi_sb[:, tj:tj + 1], axis=0))
                kq = sb.tile([P, HID], F32, tag="kq")
                nc.vector.tensor_mul(kq[:], kvg[:, 0:HID], qg[:])
                lg = sb.tile([P, H], F32, tag="lg")
                nc.vector.tensor_reduce(
                    out=lg[:], in_=kq[:].rearrange("p (h d) -> p h d", h=H),
                    axis=mybir.AxisListType.X, op=ALU.add)
                rhs = sb.tile([P, 264], F32, tag="ms264")
                nc.scalar.activation(out=rhs[:, 0:H], in_=lg[:], func=AF.Exp)
                nc.vector.tensor_tensor(
                    out=rhs[:, H:264].rearrange("p (h d) -> p h d", h=H),
                    in0=kvg[:, HID:512].rearrange("p (h d) -> p h d", h=H),
                    in1=rhs[:, 0:H].rearrange("p (h o) -> p h o", h=H)
                        .to_broadcast([P, H, DH]),
                    op=ALU.mult)
                kt_ps = pp.tile([P, P], F32, tag="tp")
                nc.tensor.transpose(
                    out=kt_ps[:],
                    in_=rkey_sb[:, tj:tj + 1].to_broadcast([P, P]),
                    identity=ident[:, :])
                kt = sb.tile([P, P], F32, tag="kt")
                nc.vector.tensor_copy(kt[:], kt_ps[:])
                sel = sb.tile([P, P], F32, tag="sel")
                nc.vector.tensor_tensor(
                    out=sel[:], in0=rkey_sb[:, tj:tj + 1].to_broadcast([P, P]),
                    in1=kt[:], op=ALU.is_equal)
                ssum = pp.tile([P, 264], F32, tag="ps")
                nc.tensor.matmul(out=ssum[:], lhsT=sel[:], rhs=rhs[:],
                                 start=True, stop=True)
                mo = sb.tile([P, 264], F32, tag="mo264")
                nc.vector.tensor_copy(mo[:], ssum[:])
                nc.gpsimd.indirect_dma_start(
                    out=part_u[l].ap(), in_=mo[:],
                    out_offset=IndirectOffsetOnAxis(
                        ap=rdst_sb[:, tj:tj + 1], axis=0),
                    in_offset=None)

            nc.gpsimd.collective_compute(
                "ReduceScatter", ALU.add, replica_groups=RG,
                ins=[part_u[l].ap()], outs=[red_u[l].ap()])

            # ---- phase E: node updates
            def fin_blend(bias_t, xs_in, xs_out, t, tag):
                og = omg[(l, t)]
                def f(ps, r0, sz):
                    ot = sb.tile([P, HID], F32, tag="do")
                    nc.vector.tensor_add(ot[0:sz], ps[0:sz], bias_t[0:sz, :])
                    xt2 = sb.tile([P, HID], F32, tag="dx2")
                    nc.sync.dma_start(xt2[0:sz], xs_in.ap()[r0:r0 + sz, :])
                    nc.vector.tensor_scalar_mul(
                        out=xt2[0:sz], in0=xt2[0:sz], scalar1=og)
                    nc.vector.tensor_add(ot[0:sz], ot[0:sz], xt2[0:sz])
                    nc.sync.dma_start(xs_out.ap()[r0:r0 + sz, :], ot[0:sz])
                return f

            # review: att rows already normalized in phase C
            def rev_att(r0, sz, tag):
                at = sb.tile([P, HID], F32, tag="ea")
                nc.sync.dma_start(at[0:sz], outs_r[l].ap()[r0:r0 + sz, :])
                nc.scalar.activation(out=at[0:sz], in_=at[0:sz], func=AF.Gelu)
                return at

            def user_att(r0, sz, tag):
                rt = sb.tile([P, 264], F32, tag="er")
                nc.sync.dma_start(rt[0:sz], red_u[l].ap()[r0:r0 + sz, :])
                nc.vector.tensor_scalar_add(
                    out=rt[0:sz, 0:H], in0=rt[0:sz, 0:H], scalar1=1e-16)
                rd = sb.tile([P, H], F32, tag="erd")
                nc.vector.reciprocal(out=rd[0:sz], in_=rt[0:sz, 0:H])
                at = sb.tile([P, HID], F32, tag="ea")
                nc.vector.tensor_tensor(
                    out=at[0:sz].rearrange("p (h d) -> p h d", h=H),
                    in0=rt[0:sz, H:264].rearrange("p (h d) -> p h d", h=H),
                    in1=rd[0:sz].rearrange("p (h o) -> p h o", h=H)
                        .to_broadcast([sz, H, DH]),
                    op=ALU.mult)
                nc.scalar.activation(out=at[0:sz], in_=at[0:sz], func=AF.Gelu)
                return at

            for t, n, attf in ((0, NU, user_att), (2, NR, rev_att)):
                wt = ws[f"wa_t{t}_l{l}"]
                fin = fin_blend(bs[f"ba_t{t}_l{l}"], xs[t][l], xs[t][l + 1],
                                t, f"e{t}{l}")
                for r0, sz in _rows_of(n):
                    at = attf(r0, sz, f"e{t}{l}")
                    xT = transposed(at, sz, HID // P, f"e{t}{l}")
                    ps = pp.tile([P, HID], F32, tag="ps")
                    for cch in range(HID // P):
                        nc.tensor.matmul(out=ps[0:sz], lhsT=xT[cch][:, 0:sz],
                                         rhs=wt[cch][:],
                                         start=(cch == 0), stop=(cch == 1))
                    fin(ps, r0, sz)
            # movie: new_x = (1-g)*x + g*ba
            og = omg[(l, 1)]
            bam = bs[f"ba_t1_l{l}"]
            for r0, sz in _rows_of(NM):
                xt = sb.tile([P, HID], F32, tag="ea")
                nc.sync.dma_start(xt[0:sz], xs[1][l].ap()[r0:r0 + sz, :])
                nc.vector.tensor_scalar_mul(out=xt[0:sz], in0=xt[0:sz],
                                            scalar1=og)
                nc.vector.tensor_add(xt[0:sz], xt[0:sz], bam[0:sz, :])
                nc.sync.dma_start(xs[1][l + 1].ap()[r0:r0 + sz, :], xt[0:sz])

        # ---- phase F: output MLP
        for t, y_dr, n in ((0, y_u, NU), (1, y_m, NM), (2, y_r, NR)):
            dense(xs[t][L], n, HID,
                  [(w2_s, fin_store(b2_s, AF.Lrelu, y_dr, 0, OUT_DIM,
                                    f"pft{t}", alpha=0.01))], f"pft{t}")
        _stk.close()

    nc.finalize()
    return nc


# ---------------------------------------------------------------- entry

_CACHE = {}


def _prepare(inputs):
    """Fold weights, pack edges, build (cached) program; return (nc, in_maps)."""
    inp = {k: np.asarray(v) for k, v in inputs.items()}
    w = _fold_weights(inp)
    T_C, cs, T_D, ru = _prep_edges(inp)
    omg = {(l, t): w[f"omg_l{l}_t{t}"] for l in range(L) for t in range(3)}

    key = (T_C, T_D)
    if key not in _CACHE:
        _CACHE[key] = build_program(T_C, T_D, omg)
    nc = _CACHE[key]

    cs_names = ["cs_src", "cs_qi", "cs_dst", "cs_key", "cs_dkey"]
    ru_names = ["ru_src", "ru_qi", "ru_dst", "ru_key"]
    in_maps = []
    for c in range(C):
        m = {
            "x_u": np.ascontiguousarray(inp["x_user"][c * NU:(c + 1) * NU]),
            "x_m": np.ascontiguousarray(inp["x_movie"][c * NM:(c + 1) * NM]),
            "x_r": np.ascontiguousarray(inp["x_review"][c * NR:(c + 1) * NR]),
            "w1": w["w1"], "b1": w["b1"], "w2": w["w2"], "b2": w["b2"],
        }
        for n, a in zip(cs_names, cs[c]):
            m[n] = a
        for n, a in zip(ru_names, ru[c]):
            m[n] = a
        for l in range(L):
            for s in range(3):
                m[f"wkv_t{s}_l{l}"] = w[f"wkv_t{s}_l{l}"]
                m[f"bkv_t{s}_l{l}"] = w[f"bkv_t{s}_l{l}"]
            for t in (0, 2):
                for nme in (f"wq_t{t}_l{l}", f"bq_t{t}_l{l}",
                            f"wa_t{t}_l{l}", f"ba_t{t}_l{l}"):
                    m[nme] = w[nme]
            m[f"ba_t1_l{l}"] = w[f"ba_t1_l{l}"]
        in_maps.append(m)
    return nc, in_maps


def kernel(**inputs):
    import os
    nc, in_maps = _prepare(inputs)
    trace = os.environ.get("BASS_KERNEL_TRACE") == "1"
    res = run_bass_kernel_spmd(nc, in_maps, core_ids=list(range(C)),
                               trace=trace)
    global LAST_RESULTS
    LAST_RESULTS = res
    r = res.results
    yu = np.concatenate([r[c]["y_u"] for c in range(C)], 0)
    ym = np.concatenate([r[c]["y_m"] for c in range(C)], 0)
    yr = np.concatenate([r[c]["y_r"] for c in range(C)], 0)
    return np.concatenate([yu, ym, yr], 0).astype(np.float32)



# revision 5
# speedup vs baseline: 1.3901x; 1.3901x over previous
"""HGT (heterogeneous graph transformer) kernel for 8 trn2 NeuronCores.

v2 design (batched gather/scatter-add, bf16, SBUF accumulators):
  - Host folds a_rel/m_rel/p_rel into per-src-type effective Wk/Wv and
    sigmoid(skip) into Wa/ba; all activations/weights bf16, PSUM f32.
  - Nodes sharded 8 ways (user 6250, movie 2500, review 25000 per core).
  - Per layer: movie/user k|v and user q AllGathered (bf16); review k|v and
    q computed locally.
  - Edge phases use dma_gather (up to 1024 edges per call, int16 indices)
    to fetch source k|v rows and dst q rows; per-edge payload
    (exp|exp*v) [264] is merged per-dst with a 128x128 selection matmul
    (edges packed no-straddle by dst), then dma_scatter_add'ed into
    parity-layout SBUF accumulators (unique real indices per call;
    duplicate writes only hit a trash row). Softmax normalization is
    deferred to a fused phase E that reads the SBUF accumulators.
  - Review-dst edges (mr, ur) are processed in 4 dst-chunks so the
    accumulators fit SBUF; ur edges are sub-grouped by src user < 32000
    (int16 index range), mr edges index the movie table globally.
  - Review->user edges grouped by dst shard; per-shard SBUF accumulators
    are dumped to DRAM and ReduceScattered (bf16) to the owner core.
  - Accumulator scatter-adds ping-pong between two sets so consecutive
    calls are not serialized by WAW on the same tile; sets are merged
    once per chunk.
"""

import math
import numpy as np

try:
    import concourse  # noqa
except ImportError:
    import sys
    sys.path.insert(0, "/opt/trn_rl_repo")

import ml_dtypes
from concourse import bacc, bass, mybir, tile
from concourse.bass_utils import run_bass_kernel_spmd
from concourse.masks import make_identity

P = 128
H, DH, HID, IN_DIM, OUT_DIM = 8, 32, 256, 768, 128
L = 2
NU_F, NM_F, NR_F = 50000, 20000, 200000
C = 8
NU, NM, NR = NU_F // C, NM_F // C, NR_F // C  # 6250, 2500, 25000
NCH = 8                    # review dst chunks per core
CSZ = NR // NCH            # 3125 reviews per chunk
ACC_ROWS = 6400            # accum rows (50 slots of 128, 25 groups)
NGRP = ACC_ROWS // 256     # 25 parity groups
TYPE_OFF = 3200            # ur rows sit above mr rows in the review accum
TRASH_R = 3150             # unused review-accum row (mr pad region)
TRASH_U = 6300             # unused user-accum row (slot 49)
EPAY = 264                 # payload elems: 8 den | 256 msg
NB = 8                     # max tiles (128 edges each) per gather/scatter call
USPLIT = 32000             # user table split point for int16 gather indices
F32 = mybir.dt.float32
BF16 = mybir.dt.bfloat16
I16 = mybir.dt.int16
I32 = mybir.dt.int32
AF = mybir.ActivationFunctionType
ALU = mybir.AluOpType
BF = ml_dtypes.bfloat16

LAST_RESULTS = None  # test.py reads exec_time_ns/profile from here


# ---------------------------------------------------------------- host prep

def _fold_weights(inp):
    """Fold a_rel/m_rel/p_rel into Wk/Wv per src type; sigmoid(skip) into Wa."""
    Wk, bk = inp["Wk"], inp["bk"]
    Wq, bq = inp["Wq"], inp["bq"]
    Wv, bv = inp["Wv"], inp["bv"]
    Wa, ba = inp["Wa"], inp["ba"]
    a_rel, m_rel, p_rel, skip = inp["a_rel"], inp["m_rel"], inp["p_rel"], inp["skip"]
    s_of_e = {0: 1, 1: 0, 2: 2}  # edge type -> src node type
    out = {}
    for l in range(L):
        for e in range(3):
            s = s_of_e[e]
            wk_eff = np.empty((HID, HID), np.float32)
            bk_eff = np.empty((HID,), np.float32)
            wv_eff = np.empty((HID, HID), np.float32)
            bv_eff = np.empty((HID,), np.float32)
            for h in range(H):
                sl = slice(h * DH, (h + 1) * DH)
                sc = float(p_rel[l, e, h]) / math.sqrt(DH)
                wk_eff[:, sl] = (Wk[l, s][:, sl] @ a_rel[l, e, h]) * sc
                bk_eff[sl] = (bk[l, s][sl] @ a_rel[l, e, h]) * sc
                wv_eff[:, sl] = Wv[l, s][:, sl] @ m_rel[l, e, h]
                bv_eff[sl] = bv[l, s][sl] @ m_rel[l, e, h]
            # kv interleaved: one gather fetches both k and v
            out[f"wkv_t{s}_l{l}"] = np.concatenate([wk_eff, wv_eff], 1).astype(BF)
            out[f"bkv_t{s}_l{l}"] = np.concatenate(
                [bk_eff, bv_eff]).reshape(1, 512).astype(np.float32)
        for t in (0, 2):  # q only needed for user and review (movie never dst)
            out[f"wq_t{t}_l{l}"] = np.ascontiguousarray(Wq[l, t]).astype(BF)
            out[f"bq_t{t}_l{l}"] = np.ascontiguousarray(
                bq[l, t]).reshape(1, HID).astype(np.float32)
        for t in range(3):
            g = 1.0 / (1.0 + math.exp(-float(skip[l, t])))
            out[f"omg_l{l}_t{t}"] = 1.0 - g  # python float, baked into program
            if t != 1:
                out[f"wa_t{t}_l{l}"] = (np.ascontiguousarray(Wa[l, t]) * g).astype(BF)
            out[f"ba_t{t}_l{l}"] = (np.ascontiguousarray(ba[l, t]) * g
                                    ).reshape(1, HID).astype(np.float32)
    out["w1"] = np.ascontiguousarray(inp["W1"]).astype(BF)
    out["b1"] = inp["b1"].reshape(1, HID).astype(np.float32)
    out["w2"] = np.ascontiguousarray(inp["W2"]).astype(BF)
    out["b2"] = inp["b2"].reshape(1, OUT_DIM).astype(np.float32)
    return out


def _pack(group_ids, kv_idx, q_idx, scat_real, trash):
    """Pack edges (sorted by group) into 128-slot tiles; groups never straddle
    a tile. Returns (T, kv[T,P], q[T,P], scat[T,P], key[T,P])."""
    n = len(group_ids)
    if n == 0:
        return 0, None
    order = np.argsort(group_ids, kind="stable")
    g = np.asarray(group_ids)[order]
    uniq, counts = np.unique(g, return_counts=True)
    ng = len(uniq)
    tile_id = np.empty(ng, np.int64)
    slot0 = np.empty(ng, np.int64)
    cur_t, fill = 0, 0
    cl = counts.tolist()
    for i in range(ng):
        c = cl[i]
        assert c <= P, f"group degree {c} > {P}"
        if fill + c > P:
            cur_t += 1
            fill = 0
        tile_id[i] = cur_t
        slot0[i] = fill
        fill += c
    T = cur_t + 1
    gi = np.repeat(np.arange(ng), counts)
    starts = np.cumsum(counts) - counts
    within = np.arange(n) - starts[gi]
    tid = tile_id[gi]
    slot = slot0[gi] + within
    kv = np.zeros((T, P), np.int16)
    qi = np.zeros((T, P), np.int16)
    sc = np.full((T, P), trash, np.int16)
    ky = np.full((T, P), -1.0, np.float32)
    kv[tid, slot] = np.asarray(kv_idx)[order].astype(np.int16)
    qi[tid, slot] = np.asarray(q_idx)[order].astype(np.int16)
    scat = np.where(within == 0, np.asarray(scat_real)[order], trash)
    sc[tid, slot] = scat.astype(np.int16)
    ky[tid, slot] = g.astype(np.float32)
    return T, (kv, qi, sc, ky)


def _wrap16(arr128):
    """[T, P] per-slot array -> wrap16 idx block [128, T*8] int16 for a call
    covering those T tiles: edge b = t*128 + slot."""
    T = arr128.shape[0]
    flat = arr128.reshape(T * P).astype(np.int16)
    w = flat.reshape(T * P // 16, 16).T  # [16, T*8]
    return np.ascontiguousarray(np.tile(w, (8, 1)))  # [128, T*8]


def _calls_of(T):
    """Split T tiles into calls of at most NB tiles."""
    calls = []
    t0 = 0
    while t0 < T:
        nt = min(NB, T - t0)
        calls.append(nt)
        t0 += nt
    return calls


def _prep_edges(inp):
    """Build per-core packed edge groups.

    Groups (same structure on every core, tile counts equalized):
      mr[k]        k in 0..NCH-1  : movie->review, dst chunk k
      ur[k][s]     s in 0,1       : user->review, dst chunk k, src lo/hi
      ru[d]        d in 0..C-1    : review->user, dst shard d
    Each group -> (T, kv[T,P], q[T,P], scat[T,P], key[T,P]).
    Returns (plan, per_core_blobs): plan maps group -> list of call tile
    counts + column offsets; blobs are the eidx/ekey arrays per core.
    """
    src_mr, dst_mr = np.asarray(inp["src_mr"]), np.asarray(inp["dst_mr"])
    src_ur, dst_ur = np.asarray(inp["src_ur"]), np.asarray(inp["dst_ur"])
    src_ru, dst_ru = np.asarray(inp["src_ru"]), np.asarray(inp["dst_ru"])

    groups = {}  # (kind, *sub) -> [per-core (T, arrays)]

    def add_group(key, per_core):
        groups[key] = per_core

    # mr: dst shard c, chunk k (scatter rows 0..CSZ-1)
    dmr = dst_mr.astype(np.int64)
    for k in range(NCH):
        per_core = []
        for c in range(C):
            m = (dmr // NR == c) & ((dmr % NR) // CSZ == k)
            dl = dmr[m] % NR
            dloc = dl - k * CSZ
            per_core.append(_pack(dloc, src_mr[m], dl, dloc, TRASH_R))
        add_group(("mr", k), per_core)

    # ur: dst shard c, chunk k, src split (scatter rows TYPE_OFF..)
    dur = dst_ur.astype(np.int64)
    sur = src_ur.astype(np.int64)
    for k in range(NCH):
        for s in range(2):
            per_core = []
            for c in range(C):
                lohi = (sur < USPLIT) if s == 0 else (sur >= USPLIT)
                m = (dur // NR == c) & ((dur % NR) // CSZ == k) & lohi
                dl = dur[m] % NR
                dloc = dl - k * CSZ
                kvi = sur[m] - (USPLIT if s == 1 else 0)
                per_core.append(_pack(dloc, kvi, dl, dloc + TYPE_OFF, TRASH_R))
            add_group(("ur", k, s), per_core)

    # ru: src shard c, dst shard d
    dru = dst_ru.astype(np.int64)
    srr = src_ru.astype(np.int64)
    for d in range(C):
        per_core = []
        for c in range(C):
            m = (srr // NR == c) & (dru // NU == d)
            dloc = dru[m] % NU
            per_core.append(_pack(dloc, srr[m] % NR, dloc, dloc, TRASH_U))
        add_group(("ru", d), per_core)

    # equalize tile counts across cores; build call plans + column blobs
    plan = {}
    col = 0        # idx column cursor (int16 cols)
    kcol = 0       # key column cursor (tiles)
    blobs_idx = [[] for _ in range(C)]
    blobs_key = [[] for _ in range(C)]
    for key, per_core in groups.items():
        T = max(t for t, _ in per_core)
        trash = TRASH_U if key[0] == "ru" else TRASH_R
        padded = []
        for t, arrs in per_core:
            kv = np.zeros((T, P), np.int16)
            qi = np.zeros((T, P), np.int16)
            sc = np.full((T, P), trash, np.int16)
            ky = np.full((T, P), -1.0, np.float32)
            if arrs is not None:
                kv[:t], qi[:t], sc[:t], ky[:t] = arrs
            padded.append((kv, qi, sc, ky))
        calls = []
        t0 = 0
        for nt in _calls_of(T):
            calls.append((nt, col, kcol + t0))
            for c in range(C):
                kv, qi, sc, ky = padded[c]
                blobs_idx[c].append(_wrap16(kv[t0:t0 + nt]))
                blobs_idx[c].append(_wrap16(qi[t0:t0 + nt]))
                blobs_idx[c].append(_wrap16(sc[t0:t0 + nt]))
            col += 3 * nt * 8
            t0 += nt
        for c in range(C):
            blobs_key[c].append(np.ascontiguousarray(padded[c][3].T))  # [P, T]
        plan[key] = calls
        kcol += T
    eidx = [np.concatenate(b, axis=1) for b in blobs_idx]   # [128, col]
    ekey = [np.concatenate(b, axis=1) for b in blobs_key]   # [128, kcol]
    return plan, col, kcol, eidx, ekey


# ---------------------------------------------------------------- device

def _rows_of(n):
    return [(r0, min(P, n - r0)) for r0 in range(0, n, P)]


def build_program(plan, idx_cols, key_cols, omg):
    """Build the 8-core SPMD Bass program."""
    nc = bacc.Bacc("TRN2", target_bir_lowering=False, debug=False,
                   enable_asserts=False, num_devices=C)
    RG = [list(range(C))]

    def din(name, shape, dt=BF16):
        return nc.dram_tensor(name, list(shape), dt, kind="ExternalInput")

    def dint(name, shape, dt=BF16, shared=False):
        return nc.dram_tensor(name, list(shape), dt, kind="Internal",
                              addr_space="Shared" if shared else "Local")

    x_u = din("x_u", (NU, IN_DIM))
    x_m = din("x_m", (NM, IN_DIM))
    x_r = din("x_r", (NR, IN_DIM))
    eidx_d = din("eidx", (P, idx_cols), I16)
    ekey_d = din("ekey", (P, key_cols), F32)
    w1 = din("w1", (IN_DIM, HID))
    b1 = din("b1", (1, HID), F32)
    w2 = din("w2", (HID, OUT_DIM))
    b2 = din("b2", (1, OUT_DIM), F32)
    wd, bd = {}, {}
    for l in range(L):
        for s in range(3):
            wd[f"wkv_t{s}_l{l}"] = din(f"wkv_t{s}_l{l}", (HID, 512))
            bd[f"bkv_t{s}_l{l}"] = din(f"bkv_t{s}_l{l}", (1, 512), F32)
        for t in (0, 2):
            wd[f"wq_t{t}_l{l}"] = din(f"wq_t{t}_l{l}", (HID, HID))
            bd[f"bq_t{t}_l{l}"] = din(f"bq_t{t}_l{l}", (1, HID), F32)
            wd[f"wa_t{t}_l{l}"] = din(f"wa_t{t}_l{l}", (HID, HID))
            bd[f"ba_t{t}_l{l}"] = din(f"ba_t{t}_l{l}", (1, HID), F32)
        bd[f"ba_t1_l{l}"] = din(f"ba_t1_l{l}", (1, HID), F32)
    y_u = nc.dram_tensor("y_u", [NU, OUT_DIM], F32, kind="ExternalOutput")
    y_m = nc.dram_tensor("y_m", [NM, OUT_DIM], F32, kind="ExternalOutput")
    y_r = nc.dram_tensor("y_r", [NR, OUT_DIM], F32, kind="ExternalOutput")

    # internal DRAM; xs[t][stage] for stage 0..2 (bf16)
    xs = {t: [dint(f"xs_t{t}_s{s}", (n, HID)) for s in range(L + 1)]
          for t, n in ((0, NU), (1, NM), (2, NR))}
    kvm_own = [dint(f"kvm_own_l{l}", (NM, 512)) for l in range(L)]
    kvu_own = [dint(f"kvu_own_l{l}", (NU, 512)) for l in range(L)]
    qu_own = [dint(f"qu_own_l{l}", (NU, HID)) for l in range(L)]
    kv_m = [dint(f"kv_m_l{l}", (NM_F, 512), shared=True) for l in range(L)]
    kv_u = [dint(f"kv_u_l{l}", (NU_F, 512), shared=True) for l in range(L)]
    q_uf = [dint(f"q_uf_l{l}", (NU_F, HID), shared=True) for l in range(L)]
    kv_ru = [dint(f"kv_ru_l{l}", (NR, 512)) for l in range(L)]
    q_r = [dint(f"q_r_l{l}", (NR, HID)) for l in range(L)]
    # phase D partials: [shard d][parity p] blocks of [128, NGRP*EPAY]
    part_u = [dint(f"part_u_l{l}", (C * 2 * P, NGRP * EPAY)) for l in range(L)]
    red_u = [dint(f"red_u_l{l}", (2 * P, NGRP * EPAY)) for l in range(L)]

    with tile.TileContext(nc) as tc:
        from contextlib import ExitStack
        _stk = ExitStack()
        wp = _stk.enter_context(tc.tile_pool(name="wp", bufs=1))

        def mk(shape, dt, name):
            return wp.tile(shape, dt, tag=name, name=name)

        identb = mk([P, P], BF16, "identb")
        make_identity(nc, identb[:, :])
        identf = mk([P, P], F32, "identf")
        make_identity(nc, identf[:, :])

        eidx_sb = mk([P, idx_cols], I16, "eidx_sb")
        nc.sync.dma_start(eidx_sb[:], eidx_d.ap()[:, :])
        ekey_sb = mk([P, key_cols], F32, "ekey_sb")
        nc.sync.dma_start(ekey_sb[:], ekey_d.ap()[:, :])

        def load_w(dr, in_dim, out_w, name):
            ts = []
            for cch in range(in_dim // P):
                t_ = mk([P, out_w], BF16, f"{name}_c{cch}")
                nc.sync.dma_start(t_[:], dr.ap()[cch * P:(cch + 1) * P, :])
                ts.append(t_)
            return ts

        def load_b(dr, w, name):
            t_ = mk([P, w], F32, name)
            nc.sync.dma_start(t_[:], dr.ap()[0:1, :].to_broadcast([P, w]))
            return t_

        w1_s = load_w(w1, IN_DIM, HID, "w1s")
        b1_s = load_b(b1, HID, "b1s")
        w2_s = load_w(w2, HID, OUT_DIM, "w2s")
        b2_s = load_b(b2, OUT_DIM, "b2s")
        ws, bs = {}, {}
        for k, dr in wd.items():
            ws[k] = load_w(dr, HID, 512 if k.startswith("wkv") else HID, k + "s")
        for k, dr in bd.items():
            bs[k] = load_b(dr, 512 if k.startswith("bkv") else HID, k + "s")

        # persistent accumulators (bf16): [set][parity] -> [128, NGRP, EPAY]
        # shared between phase C (mr rows 0..3124, ur rows 3200..6324) and
        # phase D (user rows 0..6249)
        acc = [[mk([P, NGRP, EPAY], BF16, f"acc_{st}_{par}")
                for par in range(2)] for st in range(2)]

        sb = _stk.enter_context(tc.tile_pool(name="sb", bufs=2))
        ep = _stk.enter_context(tc.tile_pool(name="ep", bufs=2))
        pp = _stk.enter_context(tc.tile_pool(name="pp", bufs=2, space="PSUM"))

        def transposed(xt_ap, sz, nch, tag):
            outs = []
            for cch in range(nch):
                tp = pp.tile([P, P], BF16, tag="tp")
                nc.tensor.transpose(
                    out=tp[:, 0:sz],
                    in_=xt_ap[0:sz, cch * P:(cch + 1) * P],
                    identity=identb[0:sz, 0:sz])
                ts = sb.tile([P, P], BF16, tag=f"dts{cch}")
                nc.vector.tensor_copy(ts[:, 0:sz], tp[:, 0:sz])
                outs.append(ts)
            return outs

        def dense(x_dr, nrows, in_dim, jobs, tag):
            """jobs: (w_tiles, finish(ps_ap, r0, sz))"""
            nch = in_dim // P
            for r0, sz in _rows_of(nrows):
                xt = sb.tile([P, in_dim], BF16, tag="dx")
                nc.sync.dma_start(xt[0:sz], x_dr.ap()[r0:r0 + sz, :])
                xT = transposed(xt, sz, nch, tag)
                for wt, finish in jobs:
                    ow = wt[0].shape[-1]
                    ps = pp.tile([P, ow], F32, tag="ps")
                    for cch in range(nch):
                        nc.tensor.matmul(out=ps[0:sz], lhsT=xT[cch][:, 0:sz],
                                         rhs=wt[cch][:],
                                         start=(cch == 0), stop=(cch == nch - 1))
                    finish(ps, r0, sz)

        def fin_store(bias_t, act, out_dr, ow, dt, tag, alpha=0.0):
            def f(ps, r0, sz):
                ot = sb.tile([P, ow], dt, tag="do" + str(dt))
                nc.vector.tensor_add(ot[0:sz], ps[0:sz], bias_t[0:sz, :])
                if act is not None:
                    nc.scalar.activation(out=ot[0:sz], in_=ot[0:sz], func=act,
                                         alpha=alpha)
                nc.sync.dma_start(out_dr.ap()[r0:r0 + sz, :], ot[0:sz])
            return f

        # ---- phase 0: input MLP
        for t, x_dr, n in ((0, x_u, NU), (1, x_m, NM), (2, x_r, NR)):
            dense(x_dr, n, IN_DIM,
                  [(w1_s, fin_store(b1_s, AF.Lrelu, xs[t][0], HID, BF16,
                                    f"p0t{t}", alpha=0.01))], f"p0t{t}")

        def edge_call(kv_tbl, q_tbl, nt, icol, kcol, a_ev, a_od, tag):
            """Process one call of nt tiles (nt*128 edges)."""
            nix = nt * P
            kvg = ep.tile([P, NB, 512], BF16, tag="kv")
            nc.gpsimd.dma_gather(
                out_ap=kvg[:, 0:nt, :], in_ap=kv_tbl,
                idxs_ap=eidx_sb[:, icol:icol + nt * 8],
                num_idxs=nix, num_idxs_reg=nix, elem_size=512)
            qg = ep.tile([P, NB, HID], BF16, tag="qg")
            nc.gpsimd.dma_gather(
                out_ap=qg[:, 0:nt, :], in_ap=q_tbl,
                idxs_ap=eidx_sb[:, icol + nt * 8:icol + nt * 16],
                num_idxs=nix, num_idxs_reg=nix, elem_size=HID)
            kq = ep.tile([P, NB, HID], BF16, tag="kq")
            nc.vector.tensor_mul(kq[:, 0:nt], kvg[:, 0:nt, 0:HID], qg[:, 0:nt])
            lg = ep.tile([P, NB, H], F32, tag="lg")
            nc.vector.tensor_reduce(
                out=lg[:, 0:nt],
                in_=kq[:, 0:nt].rearrange("p t (h d) -> p t h d", h=H),
                axis=mybir.AxisListType.X, op=ALU.add)
            ex = ep.tile([P, NB, H], BF16, tag="ex")
            nc.scalar.activation(out=ex[:, 0:nt], in_=lg[:, 0:nt], func=AF.Exp)
            pay = ep.tile([P, NB, EPAY], BF16, tag="pay")
            nc.vector.tensor_copy(pay[:, 0:nt, 0:H], ex[:, 0:nt])
            nc.vector.tensor_tensor(
                out=pay[:, 0:nt, H:EPAY].rearrange("p t (h d) -> p t h d", h=H),
                in0=kvg[:, 0:nt, HID:512].rearrange("p t (h d) -> p t h d", h=H),
                in1=ex[:, 0:nt].rearrange("p t (h o) -> p t h o", h=H)
                    .to_broadcast([P, nt, H, DH]),
                op=ALU.mult)
            mg = ep.tile([P, NB, EPAY], BF16, tag="mg")
            for t_ in range(nt):
                ktp = pp.tile([P, P], F32, tag="ktp")
                kc = kcol + t_
                nc.tensor.transpose(
                    out=ktp[:],
                    in_=ekey_sb[:, kc:kc + 1].to_broadcast([P, P]),
                    identity=identf[:, :])
                sel = ep.tile([P, P], BF16, tag="sel")
                nc.vector.tensor_tensor(
                    out=sel[:], in0=ekey_sb[:, kc:kc + 1].to_broadcast([P, P]),
                    in1=ktp[:], op=ALU.is_equal)
                ms = pp.tile([P, EPAY], F32, tag="ms")
                nc.tensor.matmul(out=ms[:], lhsT=sel[:], rhs=pay[:, t_, :],
                                 start=True, stop=True)
                nc.vector.tensor_copy(mg[:, t_, :], ms[:])
            nc.gpsimd.dma_scatter_add(
                out_ap=a_ev[:], in_ap=mg[:, 0:nt, :],
                idxs_ap=eidx_sb[:, icol + nt * 16:icol + nt * 24],
                num_idxs=nix, num_idxs_reg=nix, elem_size=EPAY,
                sbuf_tokens_per_rank=P, parity_reg=0, out_ap_other=a_od[:])

        def run_group(key, kv_tbl, q_tbl, cnt):
            """Emit all calls of a group, ping-ponging accumulator sets.
            cnt is the running call counter for the current accum cycle."""
            for (nt, icol, kcol) in plan[key]:
                st = cnt % 2
                a_ev, a_od = acc[st]
                edge_call(kv_tbl, q_tbl, nt, icol, kcol, a_ev, a_od, f"{key}")
                cnt += 1
            return cnt

        def zero_acc():
            for st in range(2):
                for par in range(2):
                    nc.vector.memset(acc[st][par][:], 0.0)

        def merge_acc():
            for par in range(2):
                nc.vector.tensor_add(acc[0][par][:], acc[0][par][:],
                                     acc[1][par][:])

        def attn_of(slot, sz, tag):
            """[sz, HID] f32 attention num/den for accum slot (128 rows)."""
            par, grp = slot & 1, slot >> 1
            a = acc[0][par]
            dn = ep.tile([P, H], F32, tag="dn")
            nc.vector.tensor_scalar_add(out=dn[0:sz], in0=a[0:sz, grp, 0:H],
                                        scalar1=1e-16)
            rd = ep.tile([P, H], F32, tag="rd")
            nc.vector.reciprocal(out=rd[0:sz], in_=dn[0:sz])
            at = ep.tile([P, HID], F32, tag="at" + tag)
            nc.vector.tensor_tensor(
                out=at[0:sz].rearrange("p (h d) -> p h d", h=H),
                in0=a[0:sz, grp, H:EPAY].rearrange("p (h d) -> p h d", h=H),
                in1=rd[0:sz].rearrange("p (h o) -> p h o", h=H)
                    .to_broadcast([sz, H, DH]),
                op=ALU.mult)
            return at

        def node_update(t, l, at_f, r0, sz, tag):
            """gelu(at) @ Wa + ba + omg*xs_old -> xs_new rows [r0, r0+sz)."""
            og = omg[(l, t)]
            ga = ep.tile([P, HID], BF16, tag="ga")
            nc.scalar.activation(out=ga[0:sz], in_=at_f[0:sz], func=AF.Gelu)
            xT = transposed(ga, sz, HID // P, tag)
            ps = pp.tile([P, HID], F32, tag="ps")
            wt = ws[f"wa_t{t}_l{l}"]
            for cch in range(HID // P):
                nc.tensor.matmul(out=ps[0:sz], lhsT=xT[cch][:, 0:sz],
                                 rhs=wt[cch][:],
                                 start=(cch == 0), stop=(cch == 1))
            ot = sb.tile([P, HID], BF16, tag="eo")
            nc.vector.tensor_add(ot[0:sz], ps[0:sz], bs[f"ba_t{t}_l{l}"][0:sz, :])
            xt2 = sb.tile([P, HID], BF16, tag="ex2")
            nc.sync.dma_start(xt2[0:sz], xs[t][l].ap()[r0:r0 + sz, :])
            nc.vector.tensor_scalar_mul(out=xt2[0:sz], in0=xt2[0:sz], scalar1=og)
            nc.vector.tensor_add(ot[0:sz], ot[0:sz], xt2[0:sz])
            nc.sync.dma_start(xs[t][l + 1].ap()[r0:r0 + sz, :], ot[0:sz])

        for l in range(L):
            # ---- phase A1: movie/user kqv (feeds the AllGathers)
            dense(xs[0][l], NU, HID, [
                (ws[f"wkv_t0_l{l}"], fin_store(bs[f"bkv_t0_l{l}"], None,
                                               kvu_own[l], 512, BF16, f"au{l}")),
                (ws[f"wq_t0_l{l}"], fin_store(bs[f"bq_t0_l{l}"], None,
                                              qu_own[l], HID, BF16, f"aq{l}")),
            ], f"au{l}")
            dense(xs[1][l], NM, HID, [
                (ws[f"wkv_t1_l{l}"], fin_store(bs[f"bkv_t1_l{l}"], None,
                                               kvm_own[l], 512, BF16, f"am{l}")),
            ], f"am{l}")

            # ---- phase B: halo allgathers (overlap with review dense)
            nc.gpsimd.collective_compute(
                "AllGather", ALU.bypass, replica_groups=RG,
                ins=[kvm_own[l].ap()], outs=[kv_m[l].ap()])
            nc.gpsimd.collective_compute(
                "AllGather", ALU.bypass, replica_groups=RG,
                ins=[kvu_own[l].ap()], outs=[kv_u[l].ap()])
            nc.gpsimd.collective_compute(
                "AllGather", ALU.bypass, replica_groups=RG,
                ins=[qu_own[l].ap()], outs=[q_uf[l].ap()])

            # ---- phase A2: review kqv (local tables)
            dense(xs[2][l], NR, HID, [
                (ws[f"wkv_t2_l{l}"], fin_store(bs[f"bkv_t2_l{l}"], None,
                                               kv_ru[l], 512, BF16, f"ar{l}")),
                (ws[f"wq_t2_l{l}"], fin_store(bs[f"bq_t2_l{l}"], None,
                                              q_r[l], HID, BF16, f"arq{l}")),
            ], f"ar{l}")

            # ---- phase C + fused review update, per dst chunk
            for k in range(NCH):
                zero_acc()
                cnt = 0
                cnt = run_group(("mr", k), kv_m[l].ap()[:, :],
                                q_r[l].ap()[:, :], cnt)
                cnt = run_group(("ur", k, 0), kv_u[l].ap()[0:USPLIT, :],
                                q_r[l].ap()[:, :], cnt)
                cnt = run_group(("ur", k, 1), kv_u[l].ap()[USPLIT:NU_F, :],
                                q_r[l].ap()[:, :], cnt)
                merge_acc()
                for b, (r0, sz) in enumerate(_rows_of(CSZ)):
                    at_m = attn_of(b, sz, "m")
                    at_u = attn_of(TYPE_OFF // P + b, sz, "u")
                    nc.vector.tensor_add(at_m[0:sz], at_m[0:sz], at_u[0:sz])
                    node_update(2, l, at_m, k * CSZ + r0, sz, f"er{l}")

            # ---- phase D: ru edges per dst shard -> partials
            for d in range(C):
                zero_acc()
                run_group(("ru", d), kv_ru[l].ap()[:, :],
                          q_uf[l].ap()[d * NU:(d + 1) * NU, :], 0)
                merge_acc()
                for par in range(2):
                    nc.sync.dma_start(
                        part_u[l].ap()[(d * 2 + par) * P:(d * 2 + par + 1) * P, :],
                        acc[0][par][:].rearrange("p g e -> p (g e)"))

            nc.gpsimd.collective_compute(
                "ReduceScatter", ALU.add, replica_groups=RG,
                ins=[part_u[l].ap()], outs=[red_u[l].ap()])

            # ---- phase E: movie blend (overlaps RS)
            og = omg[(l, 1)]
            bam = bs[f"ba_t1_l{l}"]
            for r0, sz in _rows_of(NM):
                xt = sb.tile([P, HID], BF16, tag="em")
                nc.sync.dma_start(xt[0:sz], xs[1][l].ap()[r0:r0 + sz, :])
                nc.vector.tensor_scalar_mul(out=xt[0:sz], in0=xt[0:sz],
                                            scalar1=og)
                nc.vector.tensor_add(xt[0:sz], xt[0:sz], bam[0:sz, :])
                nc.sync.dma_start(xs[1][l + 1].ap()[r0:r0 + sz, :], xt[0:sz])

            # ---- phase E: user update from red_u
            for b, (r0, sz) in enumerate(_rows_of(NU)):
                par, grp = b & 1, b >> 1
                ru_t = ep.tile([P, EPAY], BF16, tag="rut")
                nc.sync.dma_start(
                    ru_t[0:sz],
                    red_u[l].ap()[par * P:par * P + sz,
                                  grp * EPAY:(grp + 1) * EPAY])
                dn = ep.tile([P, H], F32, tag="dn")
                nc.vector.tensor_scalar_add(out=dn[0:sz], in0=ru_t[0:sz, 0:H],
                                            scalar1=1e-16)
                rd = ep.tile([P, H], F32, tag="rd")
                nc.vector.reciprocal(out=rd[0:sz], in_=dn[0:sz])
                at = ep.tile([P, HID], F32, tag="atU")
                nc.vector.tensor_tensor(
                    out=at[0:sz].rearrange("p (h d) -> p h d", h=H),
                    in0=ru_t[0:sz, H:EPAY].rearrange("p (h d) -> p h d", h=H),
                    in1=rd[0:sz].rearrange("p (h o) -> p h o", h=H)
                        .to_broadcast([sz, H, DH]),
                    op=ALU.mult)
                node_update(0, l, at, r0, sz, f"eu{l}")

        # ---- phase F: output MLP
        for t, y_dr, n in ((0, y_u, NU), (1, y_m, NM), (2, y_r, NR)):
            dense(xs[t][L], n, HID,
                  [(w2_s, fin_store(b2_s, AF.Lrelu, y_dr, OUT_DIM, F32,
                                    f"pft{t}", alpha=0.01))], f"pft{t}")
        _stk.close()

    nc.finalize()
    return nc


# ---------------------------------------------------------------- entry

_CACHE = {}


def _prepare(inputs):
    """Fold weights, pack edges, build (cached) program; return (nc, in_maps)."""
    inp = {k: np.asarray(v) for k, v in inputs.items()}
    w = _fold_weights(inp)
    plan, idx_cols, key_cols, eidx, ekey = _prep_edges(inp)
    omg = {(l, t): w[f"omg_l{l}_t{t}"] for l in range(L) for t in range(3)}

    key = (tuple(sorted((k, tuple(v)) for k, v in plan.items())),
           idx_cols, key_cols)
    if key not in _CACHE:
        _CACHE[key] = build_program(plan, idx_cols, key_cols, omg)
    nc = _CACHE[key]

    xu = inp["x_user"].astype(BF)
    xm = inp["x_movie"].astype(BF)
    xr = inp["x_review"].astype(BF)
    in_maps = []
    for c in range(C):
        m = {
            "x_u": np.ascontiguousarray(xu[c * NU:(c + 1) * NU]),
            "x_m": np.ascontiguousarray(xm[c * NM:(c + 1) * NM]),
            "x_r": np.ascontiguousarray(xr[c * NR:(c + 1) * NR]),
            "eidx": eidx[c], "ekey": ekey[c],
            "w1": w["w1"], "b1": w["b1"], "w2": w["w2"], "b2": w["b2"],
        }
        for l in range(L):
            for s in range(3):
                m[f"wkv_t{s}_l{l}"] = w[f"wkv_t{s}_l{l}"]
                m[f"bkv_t{s}_l{l}"] = w[f"bkv_t{s}_l{l}"]
            for t in (0, 2):
                for nme in (f"wq_t{t}_l{l}", f"bq_t{t}_l{l}",
                            f"wa_t{t}_l{l}", f"ba_t{t}_l{l}"):
                    m[nme] = w[nme]
            m[f"ba_t1_l{l}"] = w[f"ba_t1_l{l}"]
        in_maps.append(m)
    return nc, in_maps


def kernel(**inputs):
    import os
    nc, in_maps = _prepare(inputs)
    trace = os.environ.get("BASS_KERNEL_TRACE") == "1"
    res = run_bass_kernel_spmd(nc, in_maps, core_ids=list(range(C)),
                               trace=trace)
    global LAST_RESULTS
    LAST_RESULTS = res
    r = res.results
    yu = np.concatenate([r[c]["y_u"] for c in range(C)], 0)
    ym = np.concatenate([r[c]["y_m"] for c in range(C)], 0)
    yr = np.concatenate([r[c]["y_r"] for c in range(C)], 0)
    return np.concatenate([yu, ym, yr], 0).astype(np.float32)
